# revision 13
# baseline (speedup 1.0000x reference)
"""Dense CRF forward (5 mean-field iterations, exact dense bilateral kernel)
on 8 Trainium2 NeuronCores via Bass/Tile.

Sharding: core c -> (batch n = c//4, group-rank g = c%4). Each core owns 1024
of the 4096 pixels of one batch element: it builds and keeps resident in SBUF
the [4096 x 1024] column-block W of the (symmetric) matrix 4*K, where
K[p,q] = exp(-0.5*||f_p - f_q||^2). Each iteration computes
  q_hat[p,c] = U[p,c] + sum_q W[q,p]*q[q,c] + 2*(spatial conv)(q)[p,c]
then softmax over c. The new q shard is exchanged among the 4 cores of the
batch group with direct peer-to-peer SBUF DMA (remote_dma_broadcast with
relative XOR dests) instead of a DRAM AllGather collective -- the collective's
~25us firmware latency floor dominated the baseline.

XOR span layout: core with group-rank g stores pixel-block (g XOR s) at qpc
span s (so span 0 is always its own block, written by the local epilogue).
Sending message m in {1,2,3} to relative dest (0, m) lands my block at peer
(g^m)'s span m -- the same compile-time span for every core, keeping the
program SPMD-uniform. fa (q-side features), q0 and the conv's ay matrix are
permuted host-side to match. qpc is double-buffered by iteration parity; the
semaphore chain makes peer writes land only after the receiver finished
reading that buffer two iterations earlier (no barriers needed).

The 71x71 depthwise Gaussian conv is exactly separable (gk is rank-1),
computed as two 64x64 Toeplitz-matrix matmul stages from a row-layout copy
qyl of q, gathered locally from the qpc spans with HWDGE DMAs.

The W build (fp32r matmul over hi/lo-split features + scalar exp) is
interleaved with iteration-0's qbf accumulation so the PE never idles (idle
gaps trip the HAM activity throttle and halve the PE clock).
"""
import os
import sys

for _p in ("/opt/trn_rl_repo", "/root/.axon_site/_ro/trn_rl_repo"):
    if os.path.isdir(_p) and _p not in sys.path:
        sys.path.insert(0, _p)

import numpy as np
import concourse.bass as bass  # noqa: E402
import concourse.bass_interp as _bi  # noqa: E402
import concourse.tile as tile  # noqa: E402
from concourse import mybir, bacc  # noqa: E402
from concourse.bass_utils import run_bass_kernel_spmd  # noqa: E402
from concourse.tile_rust import add_dep_helper  # noqa: E402

# The Tile scheduler's single-core virtual sim cannot model peer-to-peer
# semaphore increments (remote_dma remote_sem updates land on other cores),
# so any wait on them would falsely deadlock the SCHEDULING pass. Pre-charge
# those semaphores in the scheduler's virtual state only -- the compiled
# program is unchanged and still waits at runtime.
_SIM_PRECHARGE: dict[int, tuple[str, int]] = {}
_orig_coresim_simulate = _bi.CoreSim.simulate


def _simulate_with_precharge(self):
    for sem_id, (sem_name, val) in _SIM_PRECHARGE.items():
        upd = mybir.SyncUpdate(
            sync_type="semaphore", id=sem_id, ant_name=sem_name,
            update_mode="sem-add-imm", update_value=val)
        self.update_semaphore(upd)
    return _orig_coresim_simulate(self)


_bi.CoreSim.simulate = _simulate_with_precharge

F32 = mybir.dt.float32
F32R = mybir.dt.float32r
EXP = mybir.ActivationFunctionType.Exp
AX = mybir.AxisListType
ALU = mybir.AluOpType

N, C, H, W_IMG = 2, 21, 64, 64
P = H * W_IMG            # 4096 pixels
NB = 4                   # blocks (cores) per batch element
PB = P // NB             # 1024 pixels per block
T = P // 128             # 32 q-tiles of 128 pixels
PC = PB // 128           # 8 p-chunks of 128 pixels per block
NUM_ITER = 5
COMPAT_BF, COMPAT_SPATIAL = 4.0, 2.0
KD = 18                  # split-feature contraction dims

# remote exchange: 3 msgs/round, each a 1-dest broadcast -> remote_sem += 2
RSEM_PER_ROUND = 6
LSEM_PER_ROUND = 48      # 3 broadcasts x local_sem += 16

TRACE = False
LAST_EXEC_NS = None
LAST_RESULTS = None

_CACHED_NC = None


def _build_program():
    nc = bacc.Bacc("TRN2", target_bir_lowering=False, debug=False, num_devices=8)

    fA_d = nc.dram_tensor("fa", [KD, P], F32R, kind="ExternalInput")
    fB_d = nc.dram_tensor("fb", [KD, PB], F32R, kind="ExternalInput")
    u_d = nc.dram_tensor("u_blk", [128, PC * C], F32, kind="ExternalInput")
    q0pc_d = nc.dram_tensor("q0pc", [128, T * C], F32R, kind="ExternalInput")
    q0yl_d = nc.dram_tensor("q0yl", [64, H * C], F32R, kind="ExternalInput")
    a2_d = nc.dram_tensor("a2mat", [64, 64], F32R, kind="ExternalInput")
    ay_d = nc.dram_tensor("ay", [64, 16], F32R, kind="ExternalInput")
    out_d = nc.dram_tensor("out_blk", [128, PC * C], F32, kind="ExternalOutput")
    id_d = nc.inline_tensor(np.eye(128, dtype=np.float32), name="ident_np")

    rsem = nc.alloc_semaphore("p2p_recv")
    lsem = nc.alloc_semaphore("p2p_sent")
    _SIM_PRECHARGE.clear()
    _SIM_PRECHARGE[rsem.num] = ("p2p_recv", RSEM_PER_ROUND * (NUM_ITER - 1))
    _SIM_PRECHARGE[lsem.num] = ("p2p_sent", LSEM_PER_ROUND * (NUM_ITER - 1))

    with tile.TileContext(nc) as tc:
        with (
            tc.tile_pool(name="const", bufs=1) as cpool,
            tc.tile_pool(name="wpool", bufs=1) as wpool,
            tc.tile_pool(name="qpool", bufs=1) as qpool,
            tc.tile_pool(name="work", bufs=1) as work,
            tc.tile_pool(name="ps_build", bufs=2, space="PSUM") as ps_build,
            tc.tile_pool(name="ps_qbf", bufs=2, space="PSUM") as ps_qbf,
            tc.tile_pool(name="ps_bt", bufs=1, space="PSUM") as ps_bt,
            tc.tile_pool(name="ps_conv", bufs=1, space="PSUM") as ps_conv,
        ):
            # ---- constants / inputs to SBUF ----
            fa = cpool.tile([KD, P], F32R, tag="fa")
            fb = cpool.tile([KD, PB], F32R, tag="fb")
            u_t = cpool.tile([128, PC * C], F32, tag="u_t")
            a2 = cpool.tile([64, 64], F32R, tag="a2")
            ay = cpool.tile([64, 16], F32R, tag="ay")
            ident = cpool.tile([128, 128], F32, tag="ident")
            nc.sync.dma_start(fa[:], fA_d[:])
            nc.sync.dma_start(fb[:], fB_d[:])
            nc.sync.dma_start(u_t[:], u_d[:])
            nc.scalar.dma_start(a2[:], a2_d[:])
            nc.scalar.dma_start(ay[:], ay_d[:])
            nc.scalar.dma_start(ident[:], id_d[:])

            # ---- persistent q tiles (f32r), double-buffered by parity ----
            qp0 = qpool.tile([128, T * C], F32R, tag="qpc0")
            qp1 = qpool.tile([128, T * C], F32R, tag="qpc1")
            qp = [qp0, qp1]
            qyl = qpool.tile([64, H * C], F32R, tag="qyl")
            nc.sync.dma_start(qp[0][:], q0pc_d[:])
            nc.sync.dma_start(qyl[:], q0yl_d[:])

            w_sb = wpool.tile([128, T * PB], F32R, tag="wsb")

            # ---- working tiles ----
            s_qbf = work.tile([C, PB], F32, tag="s_qbf")
            s_t2 = work.tile([64, C * 16], F32R, tag="s_t2")
            t0 = work.tile([128, PC * C], F32, tag="t0")
            e_t = work.tile([128, PC * C], F32, tag="e_t")
            ssum = work.tile([128, PC], F32, tag="ssum")
            rsum = work.tile([128, PC], F32, tag="rsum")
            qout = work.tile([128, PC * C], F32, tag="qout")

            def emit_qbf(it):
                """qbf accumulation for iteration `it`; returns psum pair.
                For it==0 the W build is interleaved tile-by-tile."""
                buf = qp[it % 2]
                pq_h0 = ps_qbf.tile([C, 512], F32, tag="pqbf")
                pq_h1 = ps_qbf.tile([C, 512], F32, tag="pqbf")
                pqs = [pq_h0, pq_h1]

                def qbf_tile(i, t, dep=None):
                    mms = []
                    for h in (0, 1):
                        mm = nc.tensor.matmul(
                            pqs[h], buf[:, t * C:(t + 1) * C],
                            w_sb[:, t * PB + h * 512: t * PB + (h + 1) * 512],
                            start=(i == 0), stop=(i == T - 1))
                        if dep is not None:
                            add_dep_helper(mm.ins, dep.ins,
                                           reason="qbf remote tile waits p2p sem")
                        mms.append(mm)
                    return mms

                if it == 0:
                    # interleave W build with the accumulation (2-tile lag)
                    for t in range(T):
                        pb = ps_build.tile([128, 1024], F32, tag="pbuild")
                        for h in (0, 1):
                            nc.tensor.matmul(
                                pb[:, h * 512:(h + 1) * 512],
                                fa[:, t * 128:(t + 1) * 128],
                                fb[:, h * 512:(h + 1) * 512],
                                start=True, stop=True)
                        nc.scalar.activation(
                            w_sb[:, t * PB:(t + 1) * PB], pb[:], EXP,
                            bias=0.0, scale=1.0)
                        if t >= 2:
                            qbf_tile(t - 2, t - 2)
                    qbf_tile(T - 2, T - 2)
                    qbf_tile(T - 1, T - 1)
                else:
                    # local span (tiles 0..7) first, then wait for peers
                    last0 = None
                    for t in range(PC):
                        last0 = qbf_tile(t, t)
                    wait_pe = nc.tensor.wait_ge(rsem, RSEM_PER_ROUND * it)
                    for mm in last0:
                        add_dep_helper(wait_pe.ins, mm.ins,
                                       reason="p2p wait after local qbf tiles")
                    for t in range(PC, T):
                        qbf_tile(t, t, dep=wait_pe)
                return pqs

            def gather_qyl(it):
                """qyl rows rebuilt from qp[it%2] spans. A single DMA cannot
                exchange partition roles SBUF->SBUF, so bounce each span
                through a DRAM scratch (contiguous copies), then gather rows
                with the DRAM->SBUF pattern. Remote spans gated on rsem."""
                buf = qp[it % 2]
                scr = nc.dram_tensor(f"qscr{it}", [NB * 128, PC * C], F32R,
                                     kind="Internal")
                # span 0 (own block): no wait
                cp0 = nc.sync.dma_start(scr[0:128, :], buf[:, 0:PC * C])
                waits = {}
                if it > 0:
                    for eng in (nc.sync, nc.scalar):
                        w = eng.wait_ge(rsem, RSEM_PER_ROUND * it)
                        add_dep_helper(w.ins, cp0.ins,
                                       reason="p2p wait after local span copy")
                        waits[eng] = w
                for s in range(1, NB):
                    eng = nc.scalar if s % 2 else nc.sync
                    cp = eng.dma_start(scr[s * 128:(s + 1) * 128, :],
                                       buf[:, s * PC * C:(s + 1) * PC * C])
                    if eng in waits:
                        add_dep_helper(cp.ins, waits[eng].ins,
                                       reason="qyl span copy waits p2p sem")
                src_y = scr[:].rearrange(
                    "(s ylo x) (pcl c) -> s ylo pcl x c", s=NB, ylo=2, c=C)
                for s in range(NB):
                    dsts = qyl[s * 16:(s + 1) * 16, :].rearrange(
                        "(pcl ylo) (x c) -> ylo pcl x c", ylo=2, c=C)
                    for ylo in range(2):
                        eng = nc.scalar if (2 * s + ylo) % 2 else nc.sync
                        eng.dma_start(dsts[ylo], src_y[s, ylo])

            for it in range(NUM_ITER):
                pqs = emit_qbf(it)
                for h in (0, 1):
                    nc.vector.tensor_copy(s_qbf[:, h * 512:(h + 1) * 512], pqs[h])

                # transpose qbf to [p, c] chunks
                pbt = ps_bt.tile([128, PC * C], F32, tag="pbt")
                for pc in range(PC):
                    nc.tensor.transpose(
                        pbt[:, pc * C:(pc + 1) * C],
                        s_qbf[:, pc * 128:(pc + 1) * 128], ident[:C, :C])

                # ---- spatial conv (exact separable): T2 then T3 ----
                pt2 = ps_conv.tile([64, C * 16], F32, tag="pconv")
                qyl_v = qyl[:].rearrange("p (x c) -> p c x", c=C)
                for ci in range(C):
                    nc.tensor.matmul(pt2[:, ci * 16:(ci + 1) * 16],
                                     qyl_v[:, ci, :], ay[:],
                                     start=True, stop=True)
                nc.vector.tensor_copy(s_t2[:], pt2[:])
                pt3 = ps_conv.tile([64, C * 16], F32, tag="pconv")
                nc.tensor.matmul(pt3[:], a2[:], s_t2[:], start=True, stop=True)

                # ---- epilogue: t0 = U + qbf + qsf; softmax over c (no max
                # subtraction: t0 <= ~55 for this problem, exp fits fp32) ----
                nc.vector.tensor_tensor(t0[:], pbt[:], u_t[:], op=ALU.add)
                t3v = pt3[:].rearrange("p (c pc ylo) -> p ylo pc c", pc=PC, ylo=2)
                for ylo in range(2):
                    dst = t0[ylo * 64:(ylo + 1) * 64, :].rearrange(
                        "p (pc c) -> p pc c", c=C)
                    nc.vector.tensor_tensor(dst, dst, t3v[:, ylo], op=ALU.add)
                nc.scalar.activation(e_t[:], t0[:], EXP, bias=0.0, scale=1.0)
                nc.vector.tensor_reduce(
                    ssum[:], e_t[:].rearrange("p (pc c) -> p pc c", c=C),
                    axis=AX.X, op=ALU.add)
                nc.vector.reciprocal(rsum[:], ssum[:])

                if it < NUM_ITER - 1:
                    nbuf = qp[(it + 1) % 2]
                    for pc in range(PC):
                        nc.vector.tensor_scalar_mul(
                            nbuf[:, pc * C:(pc + 1) * C],
                            e_t[:, pc * C:(pc + 1) * C], rsum[:, pc:pc + 1])
                    # p2p: my new block (span 0 of nbuf) -> peer (g^m)'s span m
                    myq = nbuf[:, 0:PC * C]
                    for m in (1, 2, 3):
                        rdests = [None] * 8
                        rdests[m - 1] = (0, m)
                        nc.gpsimd.remote_dma_broadcast(
                            nbuf[:, m * PC * C:(m + 1) * PC * C], myq,
                            rsem, lsem, rdests=rdests)
                    nc.gpsimd.trigger_dma(count=None)
                    gather_qyl(it + 1)
                else:
                    for pc in range(PC):
                        nc.vector.tensor_scalar_mul(
                            qout[:, pc * C:(pc + 1) * C],
                            e_t[:, pc * C:(pc + 1) * C], rsum[:, pc:pc + 1])
                    out_dma = nc.sync.dma_start(out_d[:], qout[:])

            # quiesce + clear the p2p semaphores so re-execution starts clean
            wq1 = nc.sync.wait_ge(rsem, RSEM_PER_ROUND * (NUM_ITER - 1))
            add_dep_helper(wq1.ins, out_dma.ins, reason="quiesce after output")
            wq2 = nc.sync.wait_ge(lsem, LSEM_PER_ROUND * (NUM_ITER - 1))
            add_dep_helper(wq2.ins, wq1.ins, reason="quiesce order")
            cl1 = nc.sync.sem_clear(rsem)
            add_dep_helper(cl1.ins, wq2.ins, reason="clear after quiesce")
            cl2 = nc.sync.sem_clear(lsem)
            add_dep_helper(cl2.ins, cl1.ins, reason="clear after quiesce")

    nc.compile()
    return nc


def _host_inputs(unary, ref, gk, kstd):
    """Build the 8 per-core input maps (fp64 host math, fp32 cast).

    XOR block layout: core with group-rank g sees pixel-block (g^s) at
    span/tile-group s. fa columns, q0pc tiles, q0yl row-groups and ay rows
    are permuted accordingly.
    """
    unary = np.asarray(unary, np.float64)
    ref = np.asarray(ref, np.float64)
    gk = np.asarray(gk, np.float64)
    kstd = np.asarray(kstd, np.float64)

    yy, xx = np.meshgrid(np.arange(H, dtype=np.float64),
                         np.arange(W_IMG, dtype=np.float64), indexing="ij")
    grid = np.broadcast_to(np.stack([yy, xx])[None], (N, 2, H, W_IMG))
    stacked = np.concatenate([grid, ref], axis=1)
    feats = (stacked / kstd[None, :, None, None]).reshape(N, 5, P)  # [N,5,P]

    # hi/lo split so every matmul operand is exact in fp32r's 11-bit mantissa
    ctr = np.array([31.5 / kstd[0], 31.5 / kstd[1],
                    127.5 / kstd[2], 127.5 / kstd[3], 127.5 / kstd[4]])
    fc = feats - ctr[None, :, None]
    fs = np.round(fc[:, :2] * 8192) / 8192          # spatial, exact on 2^-13 grid
    hh = np.round(fc[:, 2:] * 64) / 64              # color hi, exact on 2^-6 grid
    ll = fc[:, 2:] - hh                             # color lo (|l| <= 2^-7)
    Feff = np.concatenate([fs, hh + ll], axis=1)
    sq = np.sum(Feff * Feff, axis=1)                # [N,P]
    ln4 = np.log(COMPAT_BF)

    U = np.log(np.clip(unary, 1e-5, 1.0)).reshape(N, C, P)
    q0 = np.exp(U - U.max(axis=1, keepdims=True))
    q0 = q0 / q0.sum(axis=1, keepdims=True)

    g2 = gk[0, 0]
    v = g2[:, 35] / np.sqrt(g2[35, 35])
    A = np.zeros((64, 64), np.float64)
    for a in range(64):
        for b in range(64):
            if abs(b - a) <= 35:
                A[a, b] = v[b - a + 35]

    in_maps = []
    for core in range(8):
        n, g = core // NB, core % NB
        blk = slice(g * PB, (g + 1) * PB)
        # pixel permutation: tile-group s covers block g^s
        perm = np.concatenate(
            [np.arange((g ^ s) * PB, (g ^ s) * PB + PB) for s in range(NB)])
        one = np.ones(P)
        Hq = np.round(-0.5 * sq[n] * 8) / 8
        Lq = -0.5 * sq[n] - Hq
        Hp = np.round((-0.5 * sq[n] + ln4) * 8) / 8
        Lp = (-0.5 * sq[n] + ln4) - Hp
        a_dims = [fs[n][0], fs[n][1]]
        b_dims = [fs[n][0], fs[n][1]]
        for ci in range(3):
            a_dims += [hh[n][ci], hh[n][ci], ll[n][ci], ll[n][ci]]
            b_dims += [hh[n][ci], ll[n][ci], hh[n][ci], ll[n][ci]]
        a_dims += [Hq, Lq, one, one]
        b_dims += [one, one, Hp, Lp]
        fa = np.stack(a_dims)[:, perm].astype(np.float32)   # [18, P] permuted
        fb = np.stack(b_dims)[:, blk].astype(np.float32)    # [18, PB]
        u_blk = (U[n].T[blk]
                 .reshape(PC, 128, C).transpose(1, 0, 2)
                 .reshape(128, PC * C).astype(np.float32))
        # q0 in permuted pixel-chunk layout and permuted row layout
        q0p = q0[n].T[perm]                                  # [P, C]
        q0pc = (q0p.reshape(T, 128, C).transpose(1, 0, 2)
                .reshape(128, T * C).astype(np.float32))
        rowperm = np.concatenate(
            [np.arange((g ^ s) * 16, (g ^ s) * 16 + 16) for s in range(NB)])
        q0yl = (q0[n].T.reshape(H, W_IMG * C)[rowperm].astype(np.float32))
        ay = A[rowperm][:, g * 16:(g + 1) * 16]
        in_maps.append({
            "fa": fa, "fb": fb, "u_blk": u_blk,
            "q0pc": q0pc, "q0yl": q0yl,
            "a2mat": (COMPAT_SPATIAL * A).astype(np.float32),
            "ay": ay.astype(np.float32),
        })
    return in_maps


def kernel(unary, ref, gk, kstd):
    global _CACHED_NC, LAST_EXEC_NS, LAST_RESULTS
    in_maps = _host_inputs(unary, ref, gk, kstd)
    if _CACHED_NC is None:
        _CACHED_NC = _build_program()
    res = run_bass_kernel_spmd(_CACHED_NC, in_maps, core_ids=list(range(8)),
                               trace=TRACE)
    LAST_EXEC_NS = res.exec_time_ns
    LAST_RESULTS = res
    q_full = np.zeros((N, P, C), np.float32)
    for core in range(8):
        n, g = core // NB, core % NB
        blk = res.results[core]["out_blk"]
        q_full[n, g * PB:(g + 1) * PB] = (
            blk.reshape(128, PC, C).transpose(1, 0, 2).reshape(PB, C))
    return q_full.transpose(0, 2, 1).reshape(N, C, H, W_IMG).astype(np.float32)


# revision 15
# speedup vs baseline: 36.4491x; 36.4491x over previous
"""Dense CRF forward (5 mean-field iterations, exact dense bilateral kernel)
on 8 Trainium2 NeuronCores via Bass/Tile.

Sharding: core c -> (batch n = c//4, group-rank g = c%4). Each core owns 1024
of the 4096 pixels of one batch element: it builds and keeps resident in SBUF
the [4096 x 1024] column-block W of the (symmetric) matrix 4*K, where
K[p,q] = exp(-0.5*||f_p - f_q||^2). Each iteration computes
  q_hat[p,c] = U[p,c] + sum_q W[q,p]*q[q,c] + 2*(spatial conv)(q)[p,c]
then softmax over c. The new q shard is exchanged among the 4 cores of the
batch group with direct peer-to-peer SBUF DMA (remote_dma_broadcast with
relative XOR dests) instead of a DRAM AllGather collective -- the collective's
~25us firmware latency floor dominated the baseline.

XOR span layout: core with group-rank g stores pixel-block (g XOR s) at qpc
span s (so span 0 is always its own block, written by the local epilogue).
Sending message m in {1,2,3} to relative dest (0, m) lands my block at peer
(g^m)'s span m -- the same compile-time span for every core, keeping the
program SPMD-uniform. fa (q-side features), q0 and the conv's ay matrix are
permuted host-side to match. qpc is double-buffered by iteration parity; the
semaphore chain makes peer writes land only after the receiver finished
reading that buffer two iterations earlier (no barriers needed).

The 71x71 depthwise Gaussian conv is exactly separable (gk is rank-1),
computed as two 64x64 Toeplitz-matrix matmul stages from a row-layout copy
qyl of q, gathered locally from the qpc spans with HWDGE DMAs.

The W build (fp32r matmul over hi/lo-split features + scalar exp) is
interleaved with iteration-0's qbf accumulation so the PE never idles (idle
gaps trip the HAM activity throttle and halve the PE clock).
"""
import os
import sys

for _p in ("/opt/trn_rl_repo", "/root/.axon_site/_ro/trn_rl_repo"):
    if os.path.isdir(_p) and _p not in sys.path:
        sys.path.insert(0, _p)

import numpy as np
import concourse.bass as bass  # noqa: E402
import concourse.bass_interp as _bi  # noqa: E402
import concourse.tile as tile  # noqa: E402
from concourse import mybir, bacc  # noqa: E402
from concourse.bass_utils import run_bass_kernel_spmd  # noqa: E402
from concourse.tile_rust import add_dep_helper  # noqa: E402

# The Tile scheduler's single-core virtual sim cannot model peer-to-peer
# semaphore increments (remote_dma remote_sem updates land on other cores),
# so any wait on them would falsely deadlock the SCHEDULING pass. Pre-charge
# those semaphores in the scheduler's virtual state only -- the compiled
# program is unchanged and still waits at runtime.
_SIM_PRECHARGE: dict[int, tuple[str, int]] = {}
_orig_coresim_simulate = _bi.CoreSim.simulate


def _simulate_with_precharge(self):
    for sem_id, (sem_name, val) in _SIM_PRECHARGE.items():
        upd = mybir.SyncUpdate(
            sync_type="semaphore", id=sem_id, ant_name=sem_name,
            update_mode="sem-add-imm", update_value=val)
        self.update_semaphore(upd)
    return _orig_coresim_simulate(self)


_bi.CoreSim.simulate = _simulate_with_precharge

F32 = mybir.dt.float32
F32R = mybir.dt.float32r
EXP = mybir.ActivationFunctionType.Exp
AX = mybir.AxisListType
ALU = mybir.AluOpType

N, C, H, W_IMG = 2, 21, 64, 64
P = H * W_IMG            # 4096 pixels
NB = 4                   # blocks (cores) per batch element
PB = P // NB             # 1024 pixels per block
T = P // 128             # 32 q-tiles of 128 pixels
PC = PB // 128           # 8 p-chunks of 128 pixels per block
NUM_ITER = 5
COMPAT_BF, COMPAT_SPATIAL = 4.0, 2.0
KD = 18                  # split-feature contraction dims

# remote exchange: 3 msgs/round, each a 1-dest broadcast -> remote_sem += 2
RSEM_PER_ROUND = 6
LSEM_PER_ROUND = 48      # 3 broadcasts x local_sem += 16

TRACE = False
LAST_EXEC_NS = None
LAST_RESULTS = None

_CACHED_NC = None


def _build_program():
    nc = bacc.Bacc("TRN2", target_bir_lowering=False, debug=False, num_devices=8)

    fA_d = nc.dram_tensor("fa", [KD, P], F32R, kind="ExternalInput")
    fB_d = nc.dram_tensor("fb", [KD, PB], F32R, kind="ExternalInput")
    u_d = nc.dram_tensor("u_blk", [128, PC * C], F32, kind="ExternalInput")
    q0pc_d = nc.dram_tensor("q0pc", [128, T * C], F32R, kind="ExternalInput")
    q0yl_d = nc.dram_tensor("q0yl", [64, H * C], F32R, kind="ExternalInput")
    a2_d = nc.dram_tensor("a2mat", [64, 64], F32R, kind="ExternalInput")
    ay_d = nc.dram_tensor("ay", [64, 16], F32R, kind="ExternalInput")
    out_d = nc.dram_tensor("out_blk", [128, PC * C], F32, kind="ExternalOutput")
    id_d = nc.inline_tensor(np.eye(128, dtype=np.float32), name="ident_np")

    rsem = nc.alloc_semaphore("p2p_recv")
    lsem = nc.alloc_semaphore("p2p_sent")
    _SIM_PRECHARGE.clear()
    _SIM_PRECHARGE[rsem.num] = ("p2p_recv", RSEM_PER_ROUND * (NUM_ITER - 1))
    _SIM_PRECHARGE[lsem.num] = ("p2p_sent", LSEM_PER_ROUND * (NUM_ITER - 1))

    with tile.TileContext(nc) as tc:
        with (
            tc.tile_pool(name="const", bufs=1) as cpool,
            tc.tile_pool(name="wpool", bufs=1) as wpool,
            tc.tile_pool(name="qpool", bufs=1) as qpool,
            tc.tile_pool(name="work", bufs=1) as work,
            tc.tile_pool(name="ps_build", bufs=2, space="PSUM") as ps_build,
            tc.tile_pool(name="ps_qbf", bufs=2, space="PSUM") as ps_qbf,
            tc.tile_pool(name="ps_bt", bufs=1, space="PSUM") as ps_bt,
            tc.tile_pool(name="ps_conv", bufs=1, space="PSUM") as ps_conv,
        ):
            # Dummy collective: a NEFF containing a collective gets its 8
            # cores gang-launched by the runtime; without one, per-core
            # dispatch skew reaches ~10ms and every p2p wait eats it. The
            # result is never read -- this exists purely for launch sync.
            sync_in = nc.dram_tensor("syncin", [2, 256], F32, kind="Internal")
            sync_out = nc.dram_tensor("syncout", [8, 256], F32, kind="Internal")
            nc.gpsimd.collective_compute(
                "AllGather", ALU.bypass,
                replica_groups=[[0, 1, 2, 3], [4, 5, 6, 7]],
                ins=[sync_in[:]], outs=[sync_out[:]])

            # ---- constants / inputs to SBUF ----
            fa = cpool.tile([KD, P], F32R, tag="fa")
            fb = cpool.tile([KD, PB], F32R, tag="fb")
            u_t = cpool.tile([128, PC * C], F32, tag="u_t")
            a2 = cpool.tile([64, 64], F32R, tag="a2")
            ay = cpool.tile([64, 16], F32R, tag="ay")
            ident = cpool.tile([128, 128], F32, tag="ident")
            nc.sync.dma_start(fa[:], fA_d[:])
            nc.sync.dma_start(fb[:], fB_d[:])
            nc.sync.dma_start(u_t[:], u_d[:])
            nc.scalar.dma_start(a2[:], a2_d[:])
            nc.scalar.dma_start(ay[:], ay_d[:])
            nc.scalar.dma_start(ident[:], id_d[:])

            # ---- persistent q tiles (f32r), double-buffered by parity ----
            qp0 = qpool.tile([128, T * C], F32R, tag="qpc0")
            qp1 = qpool.tile([128, T * C], F32R, tag="qpc1")
            qp = [qp0, qp1]
            qyl = qpool.tile([64, H * C], F32R, tag="qyl")
            nc.sync.dma_start(qp[0][:], q0pc_d[:])
            nc.sync.dma_start(qyl[:], q0yl_d[:])

            w_sb = wpool.tile([128, T * PB], F32R, tag="wsb")

            # ---- working tiles ----
            s_qbf = work.tile([C, PB], F32, tag="s_qbf")
            s_t2 = work.tile([64, C * 16], F32R, tag="s_t2")
            t0 = work.tile([128, PC * C], F32, tag="t0")
            e_t = work.tile([128, PC * C], F32, tag="e_t")
            ssum = work.tile([128, PC], F32, tag="ssum")
            rsum = work.tile([128, PC], F32, tag="rsum")
            qout = work.tile([128, PC * C], F32, tag="qout")

            def emit_qbf(it):
                """qbf accumulation for iteration `it`; returns psum pair.
                For it==0 the W build is interleaved tile-by-tile."""
                buf = qp[it % 2]
                pq_h0 = ps_qbf.tile([C, 512], F32, tag="pqbf")
                pq_h1 = ps_qbf.tile([C, 512], F32, tag="pqbf")
                pqs = [pq_h0, pq_h1]

                def qbf_tile(i, t, dep=None):
                    mms = []
                    for h in (0, 1):
                        mm = nc.tensor.matmul(
                            pqs[h], buf[:, t * C:(t + 1) * C],
                            w_sb[:, t * PB + h * 512: t * PB + (h + 1) * 512],
                            start=(i == 0), stop=(i == T - 1))
                        if dep is not None:
                            add_dep_helper(mm.ins, dep.ins,
                                           reason="qbf remote tile waits p2p sem")
                        mms.append(mm)
                    return mms

                if it == 0:
                    # interleave W build with the accumulation (2-tile lag)
                    for t in range(T):
                        pb = ps_build.tile([128, 1024], F32, tag="pbuild")
                        for h in (0, 1):
                            nc.tensor.matmul(
                                pb[:, h * 512:(h + 1) * 512],
                                fa[:, t * 128:(t + 1) * 128],
                                fb[:, h * 512:(h + 1) * 512],
                                start=True, stop=True)
                        nc.scalar.activation(
                            w_sb[:, t * PB:(t + 1) * PB], pb[:], EXP,
                            bias=0.0, scale=1.0)
                        if t >= 2:
                            qbf_tile(t - 2, t - 2)
                    qbf_tile(T - 2, T - 2)
                    qbf_tile(T - 1, T - 1)
                else:
                    # local span (tiles 0..7) first, then wait for peers
                    last0 = None
                    for t in range(PC):
                        last0 = qbf_tile(t, t)
                    wait_pe = nc.tensor.wait_ge(rsem, RSEM_PER_ROUND * it)
                    for mm in last0:
                        add_dep_helper(wait_pe.ins, mm.ins,
                                       reason="p2p wait after local qbf tiles")
                    for t in range(PC, T):
                        qbf_tile(t, t, dep=wait_pe)
                return pqs

            def gather_qyl(it):
                """qyl rows rebuilt from qp[it%2] spans. A single DMA cannot
                exchange partition roles SBUF->SBUF, so bounce each span
                through a DRAM scratch (contiguous copies), then gather rows
                with the DRAM->SBUF pattern. Remote spans gated on rsem."""
                buf = qp[it % 2]
                scr = nc.dram_tensor(f"qscr{it}", [NB * 128, PC * C], F32R,
                                     kind="Internal")
                # span 0 (own block): no wait
                cp0 = nc.sync.dma_start(scr[0:128, :], buf[:, 0:PC * C])
                waits = {}
                if it > 0:
                    for eng in (nc.sync, nc.scalar):
                        w = eng.wait_ge(rsem, RSEM_PER_ROUND * it)
                        add_dep_helper(w.ins, cp0.ins,
                                       reason="p2p wait after local span copy")
                        waits[eng] = w
                for s in range(1, NB):
                    eng = nc.scalar if s % 2 else nc.sync
                    cp = eng.dma_start(scr[s * 128:(s + 1) * 128, :],
                                       buf[:, s * PC * C:(s + 1) * PC * C])
                    if eng in waits:
                        add_dep_helper(cp.ins, waits[eng].ins,
                                       reason="qyl span copy waits p2p sem")
                src_y = scr[:].rearrange(
                    "(s ylo x) (pcl c) -> s ylo pcl x c", s=NB, ylo=2, c=C)
                for s in range(NB):
                    dsts = qyl[s * 16:(s + 1) * 16, :].rearrange(
                        "(pcl ylo) (x c) -> ylo pcl x c", ylo=2, c=C)
                    for ylo in range(2):
                        eng = nc.scalar if (2 * s + ylo) % 2 else nc.sync
                        eng.dma_start(dsts[ylo], src_y[s, ylo])

            for it in range(NUM_ITER):
                pqs = emit_qbf(it)
                for h in (0, 1):
                    nc.vector.tensor_copy(s_qbf[:, h * 512:(h + 1) * 512], pqs[h])

                # transpose qbf to [p, c] chunks
                pbt = ps_bt.tile([128, PC * C], F32, tag="pbt")
                for pc in range(PC):
                    nc.tensor.transpose(
                        pbt[:, pc * C:(pc + 1) * C],
                        s_qbf[:, pc * 128:(pc + 1) * 128], ident[:C, :C])

                # ---- spatial conv (exact separable): T2 then T3 ----
                pt2 = ps_conv.tile([64, C * 16], F32, tag="pconv")
                qyl_v = qyl[:].rearrange("p (x c) -> p c x", c=C)
                for ci in range(C):
                    nc.tensor.matmul(pt2[:, ci * 16:(ci + 1) * 16],
                                     qyl_v[:, ci, :], ay[:],
                                     start=True, stop=True)
                nc.vector.tensor_copy(s_t2[:], pt2[:])
                pt3 = ps_conv.tile([64, C * 16], F32, tag="pconv")
                nc.tensor.matmul(pt3[:], a2[:], s_t2[:], start=True, stop=True)

                # ---- epilogue: t0 = U + qbf + qsf; softmax over c (no max
                # subtraction: t0 <= ~55 for this problem, exp fits fp32) ----
                nc.vector.tensor_tensor(t0[:], pbt[:], u_t[:], op=ALU.add)
                t3v = pt3[:].rearrange("p (c pc ylo) -> p ylo pc c", pc=PC, ylo=2)
                for ylo in range(2):
                    dst = t0[ylo * 64:(ylo + 1) * 64, :].rearrange(
                        "p (pc c) -> p pc c", c=C)
                    nc.vector.tensor_tensor(dst, dst, t3v[:, ylo], op=ALU.add)
                nc.scalar.activation(e_t[:], t0[:], EXP, bias=0.0, scale=1.0)
                nc.vector.tensor_reduce(
                    ssum[:], e_t[:].rearrange("p (pc c) -> p pc c", c=C),
                    axis=AX.X, op=ALU.add)
                nc.vector.reciprocal(rsum[:], ssum[:])

                if it < NUM_ITER - 1:
                    nbuf = qp[(it + 1) % 2]
                    for pc in range(PC):
                        nc.vector.tensor_scalar_mul(
                            nbuf[:, pc * C:(pc + 1) * C],
                            e_t[:, pc * C:(pc + 1) * C], rsum[:, pc:pc + 1])
                    # p2p: my new block (span 0 of nbuf) -> peer (g^m)'s span m
                    myq = nbuf[:, 0:PC * C]
                    for m in (1, 2, 3):
                        rdests = [None] * 8
                        rdests[m - 1] = (0, m)
                        nc.gpsimd.remote_dma_broadcast(
                            nbuf[:, m * PC * C:(m + 1) * PC * C], myq,
                            rsem, lsem, rdests=rdests)
                    nc.gpsimd.trigger_dma(count=None)
                    gather_qyl(it + 1)
                else:
                    for pc in range(PC):
                        nc.vector.tensor_scalar_mul(
                            qout[:, pc * C:(pc + 1) * C],
                            e_t[:, pc * C:(pc + 1) * C], rsum[:, pc:pc + 1])
                    out_dma = nc.sync.dma_start(out_d[:], qout[:])

            # quiesce + clear the p2p semaphores so re-execution starts clean
            wq1 = nc.sync.wait_ge(rsem, RSEM_PER_ROUND * (NUM_ITER - 1))
            add_dep_helper(wq1.ins, out_dma.ins, reason="quiesce after output")
            wq2 = nc.sync.wait_ge(lsem, LSEM_PER_ROUND * (NUM_ITER - 1))
            add_dep_helper(wq2.ins, wq1.ins, reason="quiesce order")
            cl1 = nc.sync.sem_clear(rsem)
            add_dep_helper(cl1.ins, wq2.ins, reason="clear after quiesce")
            cl2 = nc.sync.sem_clear(lsem)
            add_dep_helper(cl2.ins, cl1.ins, reason="clear after quiesce")

    nc.compile()
    return nc


def _host_inputs(unary, ref, gk, kstd):
    """Build the 8 per-core input maps (fp64 host math, fp32 cast).

    XOR block layout: core with group-rank g sees pixel-block (g^s) at
    span/tile-group s. fa columns, q0pc tiles, q0yl row-groups and ay rows
    are permuted accordingly.
    """
    unary = np.asarray(unary, np.float64)
    ref = np.asarray(ref, np.float64)
    gk = np.asarray(gk, np.float64)
    kstd = np.asarray(kstd, np.float64)

    yy, xx = np.meshgrid(np.arange(H, dtype=np.float64),
                         np.arange(W_IMG, dtype=np.float64), indexing="ij")
    grid = np.broadcast_to(np.stack([yy, xx])[None], (N, 2, H, W_IMG))
    stacked = np.concatenate([grid, ref], axis=1)
    feats = (stacked / kstd[None, :, None, None]).reshape(N, 5, P)  # [N,5,P]

    # hi/lo split so every matmul operand is exact in fp32r's 11-bit mantissa
    ctr = np.array([31.5 / kstd[0], 31.5 / kstd[1],
                    127.5 / kstd[2], 127.5 / kstd[3], 127.5 / kstd[4]])
    fc = feats - ctr[None, :, None]
    fs = np.round(fc[:, :2] * 8192) / 8192          # spatial, exact on 2^-13 grid
    hh = np.round(fc[:, 2:] * 64) / 64              # color hi, exact on 2^-6 grid
    ll = fc[:, 2:] - hh                             # color lo (|l| <= 2^-7)
    Feff = np.concatenate([fs, hh + ll], axis=1)
    sq = np.sum(Feff * Feff, axis=1)                # [N,P]
    ln4 = np.log(COMPAT_BF)

    U = np.log(np.clip(unary, 1e-5, 1.0)).reshape(N, C, P)
    q0 = np.exp(U - U.max(axis=1, keepdims=True))
    q0 = q0 / q0.sum(axis=1, keepdims=True)

    g2 = gk[0, 0]
    v = g2[:, 35] / np.sqrt(g2[35, 35])
    A = np.zeros((64, 64), np.float64)
    for a in range(64):
        for b in range(64):
            if abs(b - a) <= 35:
                A[a, b] = v[b - a + 35]

    in_maps = []
    for core in range(8):
        n, g = core // NB, core % NB
        blk = slice(g * PB, (g + 1) * PB)
        # pixel permutation: tile-group s covers block g^s
        perm = np.concatenate(
            [np.arange((g ^ s) * PB, (g ^ s) * PB + PB) for s in range(NB)])
        one = np.ones(P)
        Hq = np.round(-0.5 * sq[n] * 8) / 8
        Lq = -0.5 * sq[n] - Hq
        Hp = np.round((-0.5 * sq[n] + ln4) * 8) / 8
        Lp = (-0.5 * sq[n] + ln4) - Hp
        a_dims = [fs[n][0], fs[n][1]]
        b_dims = [fs[n][0], fs[n][1]]
        for ci in range(3):
            a_dims += [hh[n][ci], hh[n][ci], ll[n][ci], ll[n][ci]]
            b_dims += [hh[n][ci], ll[n][ci], hh[n][ci], ll[n][ci]]
        a_dims += [Hq, Lq, one, one]
        b_dims += [one, one, Hp, Lp]
        fa = np.stack(a_dims)[:, perm].astype(np.float32)   # [18, P] permuted
        fb = np.stack(b_dims)[:, blk].astype(np.float32)    # [18, PB]
        u_blk = (U[n].T[blk]
                 .reshape(PC, 128, C).transpose(1, 0, 2)
                 .reshape(128, PC * C).astype(np.float32))
        # q0 in permuted pixel-chunk layout and permuted row layout
        q0p = q0[n].T[perm]                                  # [P, C]
        q0pc = (q0p.reshape(T, 128, C).transpose(1, 0, 2)
                .reshape(128, T * C).astype(np.float32))
        rowperm = np.concatenate(
            [np.arange((g ^ s) * 16, (g ^ s) * 16 + 16) for s in range(NB)])
        q0yl = (q0[n].T.reshape(H, W_IMG * C)[rowperm].astype(np.float32))
        ay = A[rowperm][:, g * 16:(g + 1) * 16]
        in_maps.append({
            "fa": fa, "fb": fb, "u_blk": u_blk,
            "q0pc": q0pc, "q0yl": q0yl,
            "a2mat": (COMPAT_SPATIAL * A).astype(np.float32),
            "ay": ay.astype(np.float32),
        })
    return in_maps


def kernel(unary, ref, gk, kstd):
    global _CACHED_NC, LAST_EXEC_NS, LAST_RESULTS
    in_maps = _host_inputs(unary, ref, gk, kstd)
    if _CACHED_NC is None:
        _CACHED_NC = _build_program()
    res = run_bass_kernel_spmd(_CACHED_NC, in_maps, core_ids=list(range(8)),
                               trace=TRACE)
    LAST_EXEC_NS = res.exec_time_ns
    LAST_RESULTS = res
    q_full = np.zeros((N, P, C), np.float32)
    for core in range(8):
        n, g = core // NB, core % NB
        blk = res.results[core]["out_blk"]
        q_full[n, g * PB:(g + 1) * PB] = (
            blk.reshape(128, PC, C).transpose(1, 0, 2).reshape(PB, C))
    return q_full.transpose(0, 2, 1).reshape(N, C, H, W_IMG).astype(np.float32)
